# revision 2
# baseline (speedup 1.0000x reference)
"""Trainium2 Bass kernel for nn_CUFLayer_83640193122985.

CUF layer: per-pixel hypernet MLP (118->32->32->32->32->2304) generates 3x3
per-channel kernels at each of 128x128 target pixels; applied to the 2x
nearest-upsampled main_input [4,64,64,256]; then 1x1 projection [256->128].

Sharding: 8-way data parallel over output rows (16 rows/core, all batches).
The hypernet runs per-core on its row slab only (recompute-local, no
collectives). The DCT feature matrix is input-independent and precomputed on
host. Matmuls run in float32r (near-fp32 precision at full PE rate); the
per-pixel multiply runs in bf16 on the vector engine (2x mode); tap and
channel accumulation rides the PE's PSUM accumulation fused with the 1x1
projection.

Self-contained: hardcodes all shapes; no sibling imports.
"""

import numpy as np
import ml_dtypes

import concourse.mybir as mybir
import concourse.tile as tile
from concourse import bacc
from concourse import bass_utils

BF16 = ml_dtypes.bfloat16
F32R = mybir.dt.float32r

K = 3
DCT_BASIS = 25
B, H_IN, W_IN, C = 4, 64, 64, 256
H_T, W_T, F_OUT = 128, 128, 128
N_CORES = 8
RPC = H_T // N_CORES  # 16 output rows per core
D_IN = 118
NPIX = RPC * W_T  # 2048 pixels per core
XROWS = RPC + 2  # 18 upsampled rows incl halo
XCOLS = W_T + 2  # 130 upsampled cols incl halo

_CACHE: dict = {}


# ----------------------------------------------------------------- host side
def _build_features():
    """feat [H_T*W_T? no: H_T, W_T, 118] fp32 — input-independent constant."""
    f = np.linspace(1.0, 2.0, DCT_BASIS).astype(np.float32)
    gh = np.linspace(0.0, 1.0, H_T).astype(np.float32)
    row_enc = np.cos(np.pi * (2.0 * gh[:, None] + 1.0) * f[None, :]).astype(np.float32)
    col_enc = row_enc  # H_T == W_T, same grid
    delta = np.concatenate(
        [
            np.broadcast_to(row_enc[:, None, :], (H_T, W_T, DCT_BASIS)),
            np.broadcast_to(col_enc[None, :, :], (H_T, W_T, DCT_BASIS)),
        ],
        axis=-1,
    )
    scale = np.array([H_T / H_IN, W_T / W_IN], np.float32)
    scale_enc = np.cos(np.pi * (2.0 * scale[:, None] + 1.0) * f[None, :]).reshape(-1)
    offs = np.arange(K, dtype=np.float32) - 1.0
    ki, kj = np.meshgrid(offs, offs, indexing="ij")
    kidx = np.stack([ki, kj], -1).reshape(K * K, 2)
    f9 = np.linspace(1.0, 1.0, 9).astype(np.float32)
    kenc = np.cos(np.pi * (2.0 * kidx[..., None] + 1.0) * f9).reshape(K * K, 18).mean(0)
    feat = np.concatenate(
        [
            delta,
            np.broadcast_to(scale_enc, (H_T, W_T, 50)),
            np.broadcast_to(kenc.astype(np.float32), (H_T, W_T, 18)),
        ],
        axis=-1,
    ).astype(np.float32)
    return feat  # [128,128,118]


def _host_prep(inputs):
    """Build per-core input maps."""
    main_input = np.asarray(inputs["main_input"], np.float32)
    feat = _CACHE.get("feat")
    if feat is None:
        feat = _CACHE["feat"] = _build_features()

    idx = np.arange(H_T) // 2
    xup_full = main_input[:, idx][:, :, idx]  # [B,128,128,C]
    xup_pad = np.pad(xup_full, ((0, 0), (1, 1), (1, 1), (0, 0)))  # [B,130,130,C]

    wproj = np.ascontiguousarray(
        np.asarray(inputs["W_proj"], np.float32).reshape(2, 128, F_OUT).transpose(1, 0, 2)
    ).astype(BF16)  # [128c, 2cc, F]
    bout = np.ascontiguousarray(
        np.asarray(inputs["b_out"], np.float32).reshape(18, 128).T
    )  # [128,18]
    shared = {
        "w1": np.asarray(inputs["W1"], np.float32),
        "w2": np.asarray(inputs["W2"], np.float32),
        "w3": np.asarray(inputs["W3"], np.float32),
        "w4": np.asarray(inputs["W4"], np.float32),
        "wout": np.asarray(inputs["W_out"], np.float32),
        "wproj": wproj,
        "b1": np.asarray(inputs["b1"], np.float32).reshape(32, 1),
        "b2": np.asarray(inputs["b2"], np.float32).reshape(32, 1),
        "b3": np.asarray(inputs["b3"], np.float32).reshape(32, 1),
        "b4": np.asarray(inputs["b4"], np.float32).reshape(32, 1),
        "bout": bout,
        "bproj": np.asarray(inputs["b_proj"], np.float32).reshape(F_OUT, 1),
    }
    in_maps = []
    for k in range(N_CORES):
        r0 = k * RPC
        slab = xup_pad[:, r0 : r0 + XROWS, :, :]  # [B,18,130,C] (padded coords)
        xup_cm = np.ascontiguousarray(slab.transpose(3, 0, 1, 2)).reshape(
            2, 128, B, XROWS, XCOLS
        ).astype(BF16)
        fslab = np.ascontiguousarray(
            feat[r0 : r0 + RPC].reshape(NPIX, D_IN).T
        )  # [118,2048]
        in_maps.append({"xup": xup_cm, "feat": fslab, **shared})
    return in_maps


def _gather(results):
    """results[k]["y"] [F,B,4,512] -> [B,H_T,W_T,F] fp32."""
    out = np.empty((B, H_T, W_T, F_OUT), np.float32)
    for k, res in enumerate(results):
        y5 = res["y"].reshape(F_OUT, B, 4, 4, 128).transpose(1, 2, 3, 4, 0)
        out[:, k * RPC : (k + 1) * RPC] = y5.reshape(B, RPC, W_T, F_OUT)
    return out


# -------------------------------------------------------------- device program
def _build_program():
    f32, bf16 = mybir.dt.float32, mybir.dt.bfloat16
    Relu = mybir.ActivationFunctionType.Relu
    Ident = mybir.ActivationFunctionType.Identity

    nc = bacc.Bacc("TRN2", target_bir_lowering=False, debug=False, num_devices=N_CORES)
    xup_d = nc.dram_tensor("xup", (2, 128, B, XROWS, XCOLS), bf16, kind="ExternalInput")
    feat_d = nc.dram_tensor("feat", (D_IN, NPIX), F32R, kind="ExternalInput")
    w1_d = nc.dram_tensor("w1", (D_IN, 32), F32R, kind="ExternalInput")
    w2_d = nc.dram_tensor("w2", (32, 32), F32R, kind="ExternalInput")
    w3_d = nc.dram_tensor("w3", (32, 32), F32R, kind="ExternalInput")
    w4_d = nc.dram_tensor("w4", (32, 32), F32R, kind="ExternalInput")
    wout_d = nc.dram_tensor("wout", (32, 2304), F32R, kind="ExternalInput")
    wproj_d = nc.dram_tensor("wproj", (128, 2, F_OUT), bf16, kind="ExternalInput")
    b1_d = nc.dram_tensor("b1", (32, 1), f32, kind="ExternalInput")
    b2_d = nc.dram_tensor("b2", (32, 1), f32, kind="ExternalInput")
    b3_d = nc.dram_tensor("b3", (32, 1), f32, kind="ExternalInput")
    b4_d = nc.dram_tensor("b4", (32, 1), f32, kind="ExternalInput")
    bout_d = nc.dram_tensor("bout", (128, 18), f32, kind="ExternalInput")
    bproj_d = nc.dram_tensor("bproj", (F_OUT, 1), f32, kind="ExternalInput")
    y_d = nc.dram_tensor("y", (F_OUT, B, 4, 512), f32, kind="ExternalOutput")

    with tile.TileContext(nc) as tc:
        with (
            tc.tile_pool(name="const", bufs=1) as const,
            tc.tile_pool(name="hbuf", bufs=2) as hbuf,
            tc.tile_pool(name="kern", bufs=2) as kern_pool,
            tc.tile_pool(name="zbuf", bufs=6) as zbuf,
            tc.tile_pool(name="ybuf", bufs=3) as ybuf,
            tc.tile_pool(name="ps_mlp", bufs=2, space="PSUM") as ps_mlp,
            tc.tile_pool(name="ps_kern", bufs=3, space="PSUM") as ps_kern,
            tc.tile_pool(name="ps_y", bufs=2, space="PSUM") as ps_y,
        ):
            xup_sb = const.tile([128, 2, B, XROWS, XCOLS], bf16)
            for cc in range(2):
                nc.sync.dma_start(xup_sb[:, cc], xup_d[cc])
            feat_sb = const.tile([D_IN, NPIX], F32R)
            nc.sync.dma_start(feat_sb, feat_d[:])
            w1_sb = const.tile([D_IN, 32], F32R)
            nc.sync.dma_start(w1_sb, w1_d[:])
            w_sb = {1: w1_sb}
            for i, wd in [(2, w2_d), (3, w3_d), (4, w4_d)]:
                w = const.tile([32, 32], F32R, tag=f"w{i}")
                nc.sync.dma_start(w, wd[:])
                w_sb[i] = w
            wout_sb = const.tile([32, 2304], F32R)
            nc.sync.dma_start(wout_sb, wout_d[:])
            wproj_sb = const.tile([128, 2, F_OUT], bf16)
            nc.sync.dma_start(wproj_sb, wproj_d[:])
            b_sb = {}
            for i, bd in [(1, b1_d), (2, b2_d), (3, b3_d), (4, b4_d)]:
                bt = const.tile([32, 1], f32, tag=f"b{i}")
                nc.sync.dma_start(bt, bd[:])
                b_sb[i] = bt
            bout_sb = const.tile([128, 18], f32)
            nc.sync.dma_start(bout_sb, bout_d[:])
            bproj_sb = const.tile([F_OUT, 1], f32)
            nc.sync.dma_start(bproj_sb, bproj_d[:])

            for rb in range(4):
                # ---- hypernet MLP for this row block (512 pixels) ----
                h = feat_sb[:, rb * 512 : (rb + 1) * 512]
                for i in range(1, 5):
                    ps = ps_mlp.tile([32, 512], f32, tag="mlp")
                    nc.tensor.matmul(
                        ps, w_sb[i], h, start=True, stop=True
                    )
                    hn = hbuf.tile([32, 512], F32R, tag=f"h{i}")
                    nc.scalar.activation(hn, ps, Relu, bias=b_sb[i], scale=1.0)
                    h = hn
                kern_tiles = []
                for m in range(18):
                    t, chalf = divmod(m, 2)
                    dj = t % 3
                    ps = ps_kern.tile([128, 512], f32, tag="kern_ps")
                    nc.tensor.matmul(
                        ps,
                        wout_sb[:, m * 128 : (m + 1) * 128],
                        h,
                        start=True,
                        stop=True,
                    )
                    ps4 = ps.rearrange("p (a b) -> p a b", a=4)
                    if dj == 1:
                        km = kern_pool.tile([128, 4, XCOLS], bf16, tag=f"kern{m}")
                        nc.vector.memset(km[:, :, 0:1], 0.0)
                        nc.vector.memset(km[:, :, 129:130], 0.0)
                        nc.scalar.activation(
                            km[:, :, 1:129], ps4, Ident, bias=bout_sb[:, m : m + 1], scale=1.0
                        )
                    else:
                        km = kern_pool.tile([128, 4, 128], bf16, tag=f"kern{m}")
                        nc.scalar.activation(
                            km, ps4, Ident, bias=bout_sb[:, m : m + 1], scale=1.0
                        )
                    kern_tiles.append(km)

                # ---- apply + projection, per batch ----
                for b in range(B):
                    yp = ps_y.tile([128, 512], f32, tag="y")
                    yp4 = yp.rearrange("p (a b) -> p a b", a=4)
                    n_mm = 0
                    for cc in range(2):
                        for t in range(9):
                            di, dj = divmod(t, 3)
                            km = kern_tiles[t * 2 + cc]
                            rows = xup_sb[:, cc, b, rb * 4 + di : rb * 4 + di + 4]
                            if dj == 1:
                                z = zbuf.tile([128, 4, XCOLS], bf16, tag="z")
                                nc.vector.tensor_mul(z, rows[:, :, 0:XCOLS], km)
                                rhs = z[:, :, 1:129]
                            else:
                                z = zbuf.tile([128, 4, 128], bf16, tag="z")
                                nc.vector.tensor_mul(z, rows[:, :, dj : dj + 128], km)
                                rhs = z[:, :, :]
                            nc.tensor.matmul(
                                yp4,
                                wproj_sb[:, cc, :],
                                rhs,
                                start=(n_mm == 0),
                                stop=(n_mm == 17),
                            )
                            n_mm += 1
                    ys = ybuf.tile([F_OUT, 512], f32, tag="ysb")
                    nc.scalar.activation(ys, yp, Ident, bias=bproj_sb, scale=1.0)
                    nc.sync.dma_start(y_d[:, b, rb], ys)

    nc.compile()
    return nc


def get_program():
    nc = _CACHE.get("nc")
    if nc is None:
        nc = _CACHE["nc"] = _build_program()
    return nc


# --------------------------------------------------------------------- entry
def kernel(**inputs) -> np.ndarray:
    nc = get_program()
    in_maps = _host_prep(inputs)
    res = bass_utils.run_bass_kernel_spmd(
        nc, in_maps, core_ids=list(range(N_CORES))
    )
    return _gather(res.results)
